# revision 22
# baseline (speedup 1.0000x reference)
"""Causal self-attention (B=2, T=2048, C=1024, H=16) on 8 TRN2 NeuronCores.

Sharding: tensor-parallel over heads (2 heads per core), batch kept whole
per core.  Each core:
  qkvT = (W_c.T @ xT + b_c)        [384, 4096]   (transposed activations)
  per (b, h):  S^T = kT.T-blocks @ qT   (causal blocks only, [kt, qt])
               P~  = exp(S^T * 1/8) * causal_mask
               y_augT = sum_kt  v_aug.T @ P~     ([65, qt]; row 64 = denom)
               yT = y_augT[0:64] / denom
  part = yT_all.T @ W_proj[rows_c]  [4096, 1024]  (row-parallel partial)
Host: out = sum_c part_c + b_proj;  k, v gathered from qkvT slices.

Matmuls run as fp32r (fp32 with 11-bit mantissa, 1 cyc/row) in a 3-term
hi/lo split (hi@hi + hi@lo + lo@hi) giving fp32-level precision.
"""

import numpy as np
from contextlib import ExitStack

import concourse.bass as bass
import concourse.mybir as mybir
import concourse.tile as tile
from concourse import bacc
from concourse.bass_utils import run_bass_kernel_spmd
from concourse.masks import make_identity

B, T, C = 2, 2048, 1024
H, D = 16, 64
N_CORES = 8
HPC = H // N_CORES          # heads per core
BT = B * T
NQ = 256                    # qkv gemm moving-tile (>=256 keeps fp32r at 1 cyc/row)
QT = 512                    # attention qt tile
KT = 128                    # attention kt block
NKT = T // KT               # 16
NQT = T // QT               # 4

r32 = mybir.dt.float32r
f32 = mybir.dt.float32
AOp = mybir.AluOpType

_CACHE = {}


def _round11(a):
    """fp32 -> fp32r (11 explicit mantissa bits, round to nearest even)."""
    b = a.view(np.uint32).astype(np.uint64)
    lsb = (b >> 12) & 1
    b = b + 0x7FF + lsb
    return (b & 0xFFFF_F000).astype(np.uint32).view(np.float32)


def _split(a):
    hi = _round11(a)
    return hi, (a - hi)


def _build():
    nc = bacc.Bacc("TRN2", target_bir_lowering=False, debug=False,
                   num_devices=N_CORES)

    xh_ap = nc.dram_tensor("xh", [C, BT], r32, kind="ExternalInput").ap()
    xl_ap = nc.dram_tensor("xl", [C, BT], r32, kind="ExternalInput").ap()
    wh_ap = nc.dram_tensor("wh", [C, 3 * HPC * D], r32, kind="ExternalInput").ap()
    wl_ap = nc.dram_tensor("wl", [C, 3 * HPC * D], r32, kind="ExternalInput").ap()
    bq_ap = nc.dram_tensor("bq", [3, 128], f32, kind="ExternalInput").ap()
    wph_ap = nc.dram_tensor("wph", [128, C], r32, kind="ExternalInput").ap()
    wpl_ap = nc.dram_tensor("wpl", [128, C], r32, kind="ExternalInput").ap()
    mk_ap = nc.dram_tensor("mk", [128, 896], f32, kind="ExternalInput").ap()
    oz_ap = nc.dram_tensor("oz", [2, 128, NKT], r32, kind="ExternalInput").ap()

    part_ap = nc.dram_tensor("part", [BT, C], f32, kind="ExternalOutput").ap()
    kv_ap = nc.dram_tensor("kv", [2, B, HPC, D, T], f32, kind="ExternalOutput").ap()

    with tile.TileContext(nc) as tc, ExitStack() as ctx:
        pool = lambda name, bufs, **kw: ctx.enter_context(
            tc.tile_pool(name=name, bufs=bufs, **kw))

        wpool = pool("w", 1)
        xpool = pool("x", 2)
        t1pool = pool("t1", 1)
        spool = pool("spl", 1)
        vpool = pool("vaug", 1)
        ppool = pool("p", 4)
        ypool = pool("y", 1)
        rpool = pool("r", 2)
        opool = pool("o", 2)
        cpool = pool("const", 1)

        ps_qkv = pool("ps_qkv", 2, space="PSUM")
        ps_s = pool("ps_s", 2, space="PSUM")
        ps_y = pool("ps_y", 1, space="PSUM")
        ps_m = pool("ps_m", 1, space="PSUM")

        # ---- prefetch first QKV x-tiles so PE starts ASAP ----
        xh_r0 = xh_ap.rearrange("(k p) t -> p k t", p=128)
        xh_pre = xpool.tile([128, C // 128, NQ], r32, tag="xh")
        nc.sync.dma_start(xh_pre[:], xh_r0[:, :, 0:NQ])
        xl_r0 = xl_ap.rearrange("(k p) t -> p k t", p=128)
        xl_pre = xpool.tile([128, C // 128, NQ], r32, tag="xl")
        nc.sync.dma_start(xl_pre[:], xl_r0[:, :, 0:NQ])

        # ---- resident constants ----
        wh_t, wl_t = [], []
        for k in range(C // 128):
            wt = wpool.tile([128, 3 * HPC * D], r32, tag=f"wh{k}")
            nc.sync.dma_start(wt[:], wh_ap[k * 128:(k + 1) * 128, :])
            wh_t.append(wt)
            wt = wpool.tile([128, 3 * HPC * D], r32, tag=f"wl{k}")
            nc.sync.dma_start(wt[:], wl_ap[k * 128:(k + 1) * 128, :])
            wl_t.append(wt)
        mask_t = cpool.tile([128, 896], f32, tag="mask")
        nc.sync.dma_start(mask_t[:], mk_ap[:])
        bias_t = cpool.tile([128, 3], f32, tag="bias")
        for m in range(3):
            nc.sync.dma_start(bias_t[:, m:m + 1],
                              bq_ap[m:m + 1, :].rearrange("o a -> a o"))
        ident = cpool.tile([128, 128], f32, tag="ident")
        make_identity(nc, ident[:])

        xh_r = xh_ap.rearrange("(k p) t -> p k t", p=128)
        xl_r = xl_ap.rearrange("(k p) t -> p k t", p=128)

        for b in range(B):
            # ---- QKV gemm for this batch: t1 = W_c.T @ xT + b ----
            t1q = t1pool.tile([128, T], f32, tag="t1q")
            t1k = t1pool.tile([128, T], f32, tag="t1k")
            t1v = t1pool.tile([128, T], f32, tag="t1v")
            t1 = [t1q, t1k, t1v]
            for n in range(T // NQ):
                col0 = b * T + n * NQ
                if b == 0 and n == 0:
                    xh_tile, xl_tile = xh_pre, xl_pre
                else:
                    xh_tile = xpool.tile([128, C // 128, NQ], r32, tag="xh")
                    nc.sync.dma_start(xh_tile[:], xh_r[:, :, col0:col0 + NQ])
                    xl_tile = xpool.tile([128, C // 128, NQ], r32, tag="xl")
                    nc.sync.dma_start(xl_tile[:], xl_r[:, :, col0:col0 + NQ])
                for m in range(3):
                    ps = ps_qkv.tile([128, NQ], f32, tag="ps_qkv")
                    msl = slice(m * 128, (m + 1) * 128)
                    i, last = 0, 3 * (C // 128) - 1
                    for k in range(C // 128):
                        for wt, xt in ((wh_t[k], xh_tile), (wh_t[k], xl_tile),
                                       (wl_t[k], xh_tile)):
                            nc.tensor.matmul(ps[:], wt[:, msl], xt[:, k, :],
                                             start=(i == 0), stop=(i == last))
                            i += 1
                    nc.vector.tensor_scalar_add(
                        t1[m][:, n * NQ:(n + 1) * NQ], ps[:], bias_t[:, m:m + 1])

            if b == 0:
                # projection weights: needed only at proj time; load late so
                # the startup DMA window stays clear
                wph_t = wpool.tile([128, C], r32, tag="wph")
                nc.sync.dma_start(wph_t[:], wph_ap[:])
                wpl_t = wpool.tile([128, C], r32, tag="wpl")
                nc.sync.dma_start(wpl_t[:], wpl_ap[:])

            # ---- k, v DMA out (transposed layout; host untransposes) ----
            for hl in range(HPC):
                rsl = slice(hl * D, (hl + 1) * D)
                nc.sync.dma_start(kv_ap[0, b, hl], t1k[rsl, :])
                nc.sync.dma_start(kv_ap[1, b, hl], t1v[rsl, :])

            # ---- hi/lo splits of qT, kT (both heads at once) ----
            qh = spool.tile([128, T], r32, tag="qh")
            nc.vector.tensor_copy(qh[:], t1q[:])
            ql = spool.tile([128, T], r32, tag="ql")
            nc.vector.tensor_tensor(ql[:], t1q[:], qh[:], AOp.subtract)
            kh = spool.tile([128, T], r32, tag="kh")
            nc.vector.tensor_copy(kh[:], t1k[:])
            kl = spool.tile([128, T], r32, tag="kl")
            nc.vector.tensor_tensor(kl[:], t1k[:], kh[:], AOp.subtract)

            # ---- v natural (PE transpose) + ones column, hi/lo ----
            vh_aug = [vpool.tile([128, NKT, D + 1], r32, tag=f"vh{h}",
                                 name=f"vh{b}_{h}") for h in range(HPC)]
            vl_aug = [vpool.tile([128, NKT, D + 1], r32, tag=f"vl{h}",
                                 name=f"vl{b}_{h}") for h in range(HPC)]
            for h in range(HPC):
                nc.sync.dma_start(vh_aug[h][:, :, D:D + 1],
                                  oz_ap[0:1].rearrange("o p k -> p k o"))
                nc.sync.dma_start(vl_aug[h][:, :, D:D + 1],
                                  oz_ap[1:2].rearrange("o p k -> p k o"))
            for kt in range(NKT):
                tp = ps_m.tile([128, 128], f32, tag="tp")
                nc.tensor.transpose(tp[:], t1v[:, kt * 128:(kt + 1) * 128],
                                    ident[:])
                for h in range(HPC):
                    csl = slice(h * D, (h + 1) * D)
                    nc.vector.tensor_copy(vh_aug[h][:, kt, 0:D], tp[:, csl])
                    nc.vector.tensor_tensor(vl_aug[h][:, kt, 0:D], tp[:, csl],
                                            vh_aug[h][:, kt, 0:D], AOp.subtract)

            # ---- attention ----
            yh = ypool.tile([128, T], r32, tag="yh")
            yl = ypool.tile([128, T], r32, tag="yl")
            for qt in range(NQT):
                qtsl = slice(qt * QT, (qt + 1) * QT)
                psy = [ps_y.tile([D + 1, QT], f32, tag=f"psy{h}",
                                 name=f"psy{b}_{qt}_{h}") for h in range(HPC)]
                nkt = (qt + 1) * (QT // KT)
                for ktb in range(nkt):
                    delta = ktb * KT - qt * QT
                    ktsl = slice(ktb * KT, (ktb + 1) * KT)
                    pss = [ps_s.tile([128, QT], f32, tag="ps_s",
                                       name=f"pss{b}_{qt}_{ktb}_{h}")
                           for h in range(HPC)]
                    rsls = [slice(h * D, (h + 1) * D) for h in range(HPC)]
                    for a_, b_, st, sp_ in ((kh, qh, True, False),
                                            (kh, ql, False, False),
                                            (kl, qh, False, True)):
                        for h in range(HPC):
                            nc.tensor.matmul(pss[h][:], a_[rsls[h], ktsl],
                                             b_[rsls[h], qtsl],
                                             start=st, stop=sp_)
                    for h in range(HPC):
                        pf = ppool.tile([128, QT], f32, tag="pf")
                        nc.scalar.activation(pf[:], pss[h][:],
                                             mybir.ActivationFunctionType.Exp,
                                             scale=0.125)
                        if delta >= 0:  # partial (diagonal) block: apply mask
                            pm = ppool.tile([128, QT], f32, tag="pm")
                            nc.vector.tensor_tensor(
                                pm[:], pf[:],
                                mask_t[:, 384 - delta:384 - delta + QT],
                                AOp.mult)
                            pf = pm
                        phi = ppool.tile([128, QT], r32, tag="phi")
                        nc.vector.tensor_copy(phi[:], pf[:])
                        plo = ppool.tile([128, QT], r32, tag="plo")
                        nc.vector.tensor_tensor(plo[:], pf[:], phi[:],
                                                AOp.subtract)
                        first = ktb == 0
                        last = ktb == nkt - 1
                        nc.tensor.matmul(psy[h][:], vh_aug[h][:, ktb, :],
                                         phi[:], start=first, stop=False)
                        nc.tensor.matmul(psy[h][:], vl_aug[h][:, ktb, :],
                                         phi[:], start=False, stop=False)
                        nc.tensor.matmul(psy[h][:], vh_aug[h][:, ktb, :],
                                         plo[:], start=False, stop=last)
                # normalize: yT = y_raw / denom, split hi/lo
                for h in range(HPC):
                    rsl = slice(h * D, (h + 1) * D)
                    rc = rpool.tile([1, QT], f32, tag="rc")
                    nc.vector.reciprocal(rc[:], psy[h][D:D + 1, :])
                    rb = rpool.tile([D, QT], f32, tag="rb")
                    nc.gpsimd.partition_broadcast(rb[:], rc[:])
                    nc.vector.tensor_tensor(yh[rsl, qtsl], psy[h][0:D, :],
                                            rb[:], AOp.mult)
                    tmp = rpool.tile([128, QT], f32, tag="tmp")
                    nc.vector.tensor_tensor(tmp[rsl, :], psy[h][0:D, :], rb[:],
                                            AOp.mult)
                    nc.vector.tensor_tensor(yl[rsl, qtsl], tmp[rsl, :],
                                            yh[rsl, qtsl], AOp.subtract)

            # ---- output projection (row-parallel partial) ----
            for m in range(T // 128):
                msl = slice(m * 128, (m + 1) * 128)
                osl = slice(b * T + m * 128, b * T + (m + 1) * 128)
                for oc in range(C // 512):
                    ocsl = slice(oc * 512, (oc + 1) * 512)
                    ps = ps_m.tile([128, 512], f32, tag="ps_o")
                    nc.tensor.matmul(ps[:], yh[:, msl], wph_t[:, ocsl],
                                     start=True, stop=False)
                    nc.tensor.matmul(ps[:], yh[:, msl], wpl_t[:, ocsl],
                                     start=False, stop=False)
                    nc.tensor.matmul(ps[:], yl[:, msl], wph_t[:, ocsl],
                                     start=False, stop=True)
                    ot = opool.tile([128, 512], f32, tag="ot")
                    nc.vector.tensor_copy(ot[:], ps[:])
                    nc.sync.dma_start(part_ap[osl, ocsl], ot[:])

    nc.compile()
    return nc


def _get_nc():
    if "nc" not in _CACHE:
        _CACHE["nc"] = _build()
    return _CACHE["nc"]


def make_in_maps(x, W_kqv, b_kqv, W_proj, b_proj):
    x = np.ascontiguousarray(np.asarray(x, np.float32))
    W_kqv = np.asarray(W_kqv, np.float32)
    b_kqv = np.asarray(b_kqv, np.float32)
    W_proj = np.asarray(W_proj, np.float32)

    xT = np.ascontiguousarray(x.reshape(BT, C).T)
    xh, xl = _split(xT)

    # causal strip: mk[p, g] = 1.0 if p + 384 <= g
    mk = (np.arange(128)[:, None] + 384 <= np.arange(896)[None, :]).astype(np.float32)
    oz = np.stack([np.ones((128, NKT), np.float32),
                   np.zeros((128, NKT), np.float32)])

    in_maps = []
    for c in range(N_CORES):
        heads = [c * HPC + i for i in range(HPC)]
        cols = np.concatenate([
            np.arange(blk * C + h * D, blk * C + (h + 1) * D)
            for blk in range(3) for h in heads])
        w_c = np.ascontiguousarray(W_kqv[:, cols])
        wh, wl = _split(w_c)
        bq = np.ascontiguousarray(b_kqv[cols].reshape(3, 128))
        rows = np.concatenate([np.arange(h * D, (h + 1) * D) for h in heads])
        wp_c = np.ascontiguousarray(W_proj[rows, :])
        wph, wpl = _split(wp_c)
        in_maps.append({
            "xh": xh, "xl": xl, "wh": wh, "wl": wl, "bq": bq,
            "wph": wph, "wpl": wpl, "mk": mk, "oz": oz,
        })
    return in_maps


def assemble(results, b_proj):
    b_proj = np.asarray(b_proj, np.float32)
    parts = np.stack([results[c]["part"] for c in range(N_CORES)])
    out = (parts.sum(axis=0) + b_proj[None, :]).reshape(B, T, C)
    k = np.empty((B, H, T, D), np.float32)
    v = np.empty((B, H, T, D), np.float32)
    for c in range(N_CORES):
        kv = results[c]["kv"]          # [2, B, HPC, D, T]
        for hl in range(HPC):
            k[:, c * HPC + hl] = kv[0, :, hl].transpose(0, 2, 1)
            v[:, c * HPC + hl] = kv[1, :, hl].transpose(0, 2, 1)
    return out, k, v


def kernel(x, W_kqv, b_kqv, W_proj, b_proj):
    nc = _get_nc()
    in_maps = make_in_maps(x, W_kqv, b_kqv, W_proj, b_proj)
    res = run_bass_kernel_spmd(nc, in_maps, list(range(N_CORES)))
    return assemble(res.results, b_proj)


# revision 23
# speedup vs baseline: 1.0316x; 1.0316x over previous
"""Causal self-attention (B=2, T=2048, C=1024, H=16) on 8 TRN2 NeuronCores.

Sharding: tensor-parallel over heads (2 heads per core), batch kept whole
per core.  Each core:
  qkvT = (W_c.T @ xT + b_c)        [384, 4096]   (transposed activations)
  per (b, h):  S^T = kT.T-blocks @ qT   (causal blocks only, [kt, qt])
               P~  = exp(S^T * 1/8) * causal_mask
               y_augT = sum_kt  v_aug.T @ P~     ([65, qt]; row 64 = denom)
               yT = y_augT[0:64] / denom
  part = yT_all.T @ W_proj[rows_c]  [4096, 1024]  (row-parallel partial)
Host: out = sum_c part_c + b_proj;  k, v gathered from qkvT slices.

Matmuls run as fp32r (fp32 with 11-bit mantissa, 1 cyc/row) in a 3-term
hi/lo split (hi@hi + hi@lo + lo@hi) giving fp32-level precision.
"""

import numpy as np
from contextlib import ExitStack

import concourse.bass as bass
import concourse.mybir as mybir
import concourse.tile as tile
from concourse import bacc
from concourse.bass_utils import run_bass_kernel_spmd
from concourse.masks import make_identity

B, T, C = 2, 2048, 1024
H, D = 16, 64
N_CORES = 8
HPC = H // N_CORES          # heads per core
BT = B * T
NQ = 256                    # qkv gemm moving-tile (>=256 keeps fp32r at 1 cyc/row)
QT = 512                    # attention qt tile
KT = 128                    # attention kt block
NKT = T // KT               # 16
NQT = T // QT               # 4

r32 = mybir.dt.float32r
f32 = mybir.dt.float32
AOp = mybir.AluOpType

_CACHE = {}


def _round11(a):
    """fp32 -> fp32r (11 explicit mantissa bits, round to nearest even)."""
    b = a.view(np.uint32).astype(np.uint64)
    lsb = (b >> 12) & 1
    b = b + 0x7FF + lsb
    return (b & 0xFFFF_F000).astype(np.uint32).view(np.float32)


def _split(a):
    hi = _round11(a)
    return hi, (a - hi)


def _build():
    nc = bacc.Bacc("TRN2", target_bir_lowering=False, debug=False,
                   num_devices=N_CORES)

    xh_ap = nc.dram_tensor("xh", [C, BT], r32, kind="ExternalInput").ap()
    xl_ap = nc.dram_tensor("xl", [C, BT], r32, kind="ExternalInput").ap()
    wh_ap = nc.dram_tensor("wh", [C, 3 * HPC * D], r32, kind="ExternalInput").ap()
    wl_ap = nc.dram_tensor("wl", [C, 3 * HPC * D], r32, kind="ExternalInput").ap()
    bq_ap = nc.dram_tensor("bq", [3, 128], f32, kind="ExternalInput").ap()
    wph_ap = nc.dram_tensor("wph", [128, C], r32, kind="ExternalInput").ap()
    wpl_ap = nc.dram_tensor("wpl", [128, C], r32, kind="ExternalInput").ap()
    mk_ap = nc.dram_tensor("mk", [128, 896], f32, kind="ExternalInput").ap()
    oz_ap = nc.dram_tensor("oz", [2, 128, NKT], r32, kind="ExternalInput").ap()

    part_ap = nc.dram_tensor("part", [BT, C], f32, kind="ExternalOutput").ap()
    kv_ap = nc.dram_tensor("kv", [2, B, HPC, D, T], f32, kind="ExternalOutput").ap()

    with tile.TileContext(nc) as tc, ExitStack() as ctx:
        pool = lambda name, bufs, **kw: ctx.enter_context(
            tc.tile_pool(name=name, bufs=bufs, **kw))

        wpool = pool("w", 1)
        xpool = pool("x", 2)
        t1pool = pool("t1", 1)
        spool = pool("spl", 1)
        vpool = pool("vaug", 1)
        ppool = pool("p", 3)
        ypool = pool("y", 1)
        rpool = pool("r", 2)
        opool = pool("o", 2)
        cpool = pool("const", 1)

        ps_qkv = pool("ps_qkv", 2, space="PSUM")
        ps_s = pool("ps_s", 2, space="PSUM")
        ps_y = pool("ps_y", 1, space="PSUM")
        ps_m = pool("ps_m", 1, space="PSUM")

        # ---- resident constants ----
        wh_t, wl_t = [], []
        for k in range(C // 128):
            wt = wpool.tile([128, 3 * HPC * D], r32, tag=f"wh{k}")
            nc.sync.dma_start(wt[:], wh_ap[k * 128:(k + 1) * 128, :])
            wh_t.append(wt)
            wt = wpool.tile([128, 3 * HPC * D], r32, tag=f"wl{k}")
            nc.sync.dma_start(wt[:], wl_ap[k * 128:(k + 1) * 128, :])
            wl_t.append(wt)
        wph_t = wpool.tile([128, C], r32, tag="wph")
        nc.sync.dma_start(wph_t[:], wph_ap[:])
        wpl_t = wpool.tile([128, C], r32, tag="wpl")
        nc.sync.dma_start(wpl_t[:], wpl_ap[:])
        mask_t = cpool.tile([128, 896], f32, tag="mask")
        nc.sync.dma_start(mask_t[:], mk_ap[:])
        bias_t = cpool.tile([128, 3], f32, tag="bias")
        for m in range(3):
            nc.sync.dma_start(bias_t[:, m:m + 1],
                              bq_ap[m:m + 1, :].rearrange("o a -> a o"))
        ident = cpool.tile([128, 128], f32, tag="ident")
        make_identity(nc, ident[:])

        xh_r = xh_ap.rearrange("(k p) t -> p k t", p=128)
        xl_r = xl_ap.rearrange("(k p) t -> p k t", p=128)

        for b in range(B):
            # ---- QKV gemm for this batch: t1 = W_c.T @ xT + b ----
            t1q = t1pool.tile([128, T], f32, tag="t1q")
            t1k = t1pool.tile([128, T], f32, tag="t1k")
            t1v = t1pool.tile([128, T], f32, tag="t1v")
            t1 = [t1q, t1k, t1v]
            for n in range(T // NQ):
                col0 = b * T + n * NQ
                xh_tile = xpool.tile([128, C // 128, NQ], r32, tag="xh")
                nc.sync.dma_start(xh_tile[:], xh_r[:, :, col0:col0 + NQ])
                xl_tile = xpool.tile([128, C // 128, NQ], r32, tag="xl")
                nc.sync.dma_start(xl_tile[:], xl_r[:, :, col0:col0 + NQ])
                for m in range(3):
                    ps = ps_qkv.tile([128, NQ], f32, tag="ps_qkv")
                    msl = slice(m * 128, (m + 1) * 128)
                    i, last = 0, 3 * (C // 128) - 1
                    for k in range(C // 128):
                        for wt, xt in ((wh_t[k], xh_tile), (wh_t[k], xl_tile),
                                       (wl_t[k], xh_tile)):
                            nc.tensor.matmul(ps[:], wt[:, msl], xt[:, k, :],
                                             start=(i == 0), stop=(i == last))
                            i += 1
                    nc.vector.tensor_scalar_add(
                        t1[m][:, n * NQ:(n + 1) * NQ], ps[:], bias_t[:, m:m + 1])

            # ---- k, v DMA out (transposed layout; host untransposes) ----
            for hl in range(HPC):
                rsl = slice(hl * D, (hl + 1) * D)
                nc.sync.dma_start(kv_ap[0, b, hl], t1k[rsl, :])
                nc.sync.dma_start(kv_ap[1, b, hl], t1v[rsl, :])

            # ---- hi/lo splits of qT, kT (both heads at once) ----
            qh = spool.tile([128, T], r32, tag="qh")
            nc.vector.tensor_copy(qh[:], t1q[:])
            ql = spool.tile([128, T], r32, tag="ql")
            nc.vector.tensor_tensor(ql[:], t1q[:], qh[:], AOp.subtract)
            kh = spool.tile([128, T], r32, tag="kh")
            nc.vector.tensor_copy(kh[:], t1k[:])
            kl = spool.tile([128, T], r32, tag="kl")
            nc.vector.tensor_tensor(kl[:], t1k[:], kh[:], AOp.subtract)

            # ---- v natural (PE transpose) + ones column, hi/lo ----
            vh_aug = [vpool.tile([128, NKT, D + 1], r32, tag=f"vh{h}",
                                 name=f"vh{b}_{h}") for h in range(HPC)]
            vl_aug = [vpool.tile([128, NKT, D + 1], r32, tag=f"vl{h}",
                                 name=f"vl{b}_{h}") for h in range(HPC)]
            for h in range(HPC):
                nc.sync.dma_start(vh_aug[h][:, :, D:D + 1],
                                  oz_ap[0:1].rearrange("o p k -> p k o"))
                nc.sync.dma_start(vl_aug[h][:, :, D:D + 1],
                                  oz_ap[1:2].rearrange("o p k -> p k o"))
            for kt in range(NKT):
                tp = ps_m.tile([128, 128], f32, tag="tp")
                nc.tensor.transpose(tp[:], t1v[:, kt * 128:(kt + 1) * 128],
                                    ident[:])
                for h in range(HPC):
                    csl = slice(h * D, (h + 1) * D)
                    nc.vector.tensor_copy(vh_aug[h][:, kt, 0:D], tp[:, csl])
                    nc.vector.tensor_tensor(vl_aug[h][:, kt, 0:D], tp[:, csl],
                                            vh_aug[h][:, kt, 0:D], AOp.subtract)

            # ---- attention ----
            yh = ypool.tile([128, T], r32, tag="yh")
            yl = ypool.tile([128, T], r32, tag="yl")
            for qt in range(NQT):
                qtsl = slice(qt * QT, (qt + 1) * QT)
                psy = [ps_y.tile([D + 1, QT], f32, tag=f"psy{h}",
                                 name=f"psy{b}_{qt}_{h}") for h in range(HPC)]
                nkt = (qt + 1) * (QT // KT)
                for ktb in range(nkt):
                    delta = ktb * KT - qt * QT
                    ktsl = slice(ktb * KT, (ktb + 1) * KT)
                    pss = [ps_s.tile([128, QT], f32, tag="ps_s",
                                       name=f"pss{b}_{qt}_{ktb}_{h}")
                           for h in range(HPC)]
                    rsls = [slice(h * D, (h + 1) * D) for h in range(HPC)]
                    for a_, b_, st, sp_ in ((kh, qh, True, False),
                                            (kh, ql, False, False),
                                            (kl, qh, False, True)):
                        for h in range(HPC):
                            nc.tensor.matmul(pss[h][:], a_[rsls[h], ktsl],
                                             b_[rsls[h], qtsl],
                                             start=st, stop=sp_)
                    for h in range(HPC):
                        pf = ppool.tile([128, QT], f32, tag="pf")
                        nc.scalar.activation(pf[:], pss[h][:],
                                             mybir.ActivationFunctionType.Exp,
                                             scale=0.125)
                        if delta >= 0:  # partial (diagonal) block: apply mask
                            pm = ppool.tile([128, QT], f32, tag="pm")
                            nc.vector.tensor_tensor(
                                pm[:], pf[:],
                                mask_t[:, 384 - delta:384 - delta + QT],
                                AOp.mult)
                            pf = pm
                        phi = ppool.tile([128, QT], r32, tag="phi")
                        nc.vector.tensor_copy(phi[:], pf[:])
                        plo = ppool.tile([128, QT], r32, tag="plo")
                        nc.vector.tensor_tensor(plo[:], pf[:], phi[:],
                                                AOp.subtract)
                        first = ktb == 0
                        last = ktb == nkt - 1
                        nc.tensor.matmul(psy[h][:], vh_aug[h][:, ktb, :],
                                         phi[:], start=first, stop=False)
                        nc.tensor.matmul(psy[h][:], vl_aug[h][:, ktb, :],
                                         phi[:], start=False, stop=False)
                        nc.tensor.matmul(psy[h][:], vh_aug[h][:, ktb, :],
                                         plo[:], start=False, stop=last)
                # normalize: yT = y_raw / denom, split hi/lo
                for h in range(HPC):
                    rsl = slice(h * D, (h + 1) * D)
                    rc = rpool.tile([1, QT], f32, tag="rc")
                    nc.vector.reciprocal(rc[:], psy[h][D:D + 1, :])
                    rb = rpool.tile([D, QT], f32, tag="rb")
                    nc.gpsimd.partition_broadcast(rb[:], rc[:])
                    nc.vector.tensor_tensor(yh[rsl, qtsl], psy[h][0:D, :],
                                            rb[:], AOp.mult)
                    tmp = rpool.tile([128, QT], f32, tag="tmp")
                    nc.vector.tensor_tensor(tmp[rsl, :], psy[h][0:D, :], rb[:],
                                            AOp.mult)
                    nc.vector.tensor_tensor(yl[rsl, qtsl], tmp[rsl, :],
                                            yh[rsl, qtsl], AOp.subtract)

            # ---- output projection (row-parallel partial) ----
            for m in range(T // 128):
                msl = slice(m * 128, (m + 1) * 128)
                osl = slice(b * T + m * 128, b * T + (m + 1) * 128)
                for oc in range(C // 512):
                    ocsl = slice(oc * 512, (oc + 1) * 512)
                    ps = ps_m.tile([128, 512], f32, tag="ps_o")
                    nc.tensor.matmul(ps[:], yh[:, msl], wph_t[:, ocsl],
                                     start=True, stop=False)
                    nc.tensor.matmul(ps[:], yh[:, msl], wpl_t[:, ocsl],
                                     start=False, stop=False)
                    nc.tensor.matmul(ps[:], yl[:, msl], wph_t[:, ocsl],
                                     start=False, stop=True)
                    ot = opool.tile([128, 512], f32, tag="ot")
                    nc.vector.tensor_copy(ot[:], ps[:])
                    nc.sync.dma_start(part_ap[osl, ocsl], ot[:])

    nc.compile()
    return nc


def _get_nc():
    if "nc" not in _CACHE:
        _CACHE["nc"] = _build()
    return _CACHE["nc"]


def make_in_maps(x, W_kqv, b_kqv, W_proj, b_proj):
    x = np.ascontiguousarray(np.asarray(x, np.float32))
    W_kqv = np.asarray(W_kqv, np.float32)
    b_kqv = np.asarray(b_kqv, np.float32)
    W_proj = np.asarray(W_proj, np.float32)

    xT = np.ascontiguousarray(x.reshape(BT, C).T)
    xh, xl = _split(xT)

    # causal strip: mk[p, g] = 1.0 if p + 384 <= g
    mk = (np.arange(128)[:, None] + 384 <= np.arange(896)[None, :]).astype(np.float32)
    oz = np.stack([np.ones((128, NKT), np.float32),
                   np.zeros((128, NKT), np.float32)])

    in_maps = []
    for c in range(N_CORES):
        heads = [c * HPC + i for i in range(HPC)]
        cols = np.concatenate([
            np.arange(blk * C + h * D, blk * C + (h + 1) * D)
            for blk in range(3) for h in heads])
        w_c = np.ascontiguousarray(W_kqv[:, cols])
        wh, wl = _split(w_c)
        bq = np.ascontiguousarray(b_kqv[cols].reshape(3, 128))
        rows = np.concatenate([np.arange(h * D, (h + 1) * D) for h in heads])
        wp_c = np.ascontiguousarray(W_proj[rows, :])
        wph, wpl = _split(wp_c)
        in_maps.append({
            "xh": xh, "xl": xl, "wh": wh, "wl": wl, "bq": bq,
            "wph": wph, "wpl": wpl, "mk": mk, "oz": oz,
        })
    return in_maps


def assemble(results, b_proj):
    b_proj = np.asarray(b_proj, np.float32)
    parts = np.stack([results[c]["part"] for c in range(N_CORES)])
    out = (parts.sum(axis=0) + b_proj[None, :]).reshape(B, T, C)
    k = np.empty((B, H, T, D), np.float32)
    v = np.empty((B, H, T, D), np.float32)
    for c in range(N_CORES):
        kv = results[c]["kv"]          # [2, B, HPC, D, T]
        for hl in range(HPC):
            k[:, c * HPC + hl] = kv[0, :, hl].transpose(0, 2, 1)
            v[:, c * HPC + hl] = kv[1, :, hl].transpose(0, 2, 1)
    return out, k, v


def kernel(x, W_kqv, b_kqv, W_proj, b_proj):
    nc = _get_nc()
    in_maps = make_in_maps(x, W_kqv, b_kqv, W_proj, b_proj)
    res = run_bass_kernel_spmd(nc, in_maps, list(range(N_CORES)))
    return assemble(res.results, b_proj)


# revision 24
# speedup vs baseline: 1.0819x; 1.0488x over previous
"""Causal self-attention (B=2, T=2048, C=1024, H=16) on 8 TRN2 NeuronCores.

Sharding: tensor-parallel over heads (2 heads per core), batch kept whole
per core.  Each core:
  qkvT = (W_c.T @ xT + b_c)        [384, 4096]   (transposed activations)
  per (b, h):  S^T = kT.T-blocks @ qT   (causal blocks only, [kt, qt])
               P~  = exp(S^T * 1/8) * causal_mask
               y_augT = sum_kt  v_aug.T @ P~     ([65, qt]; row 64 = denom)
               yT = y_augT[0:64] / denom
  part = yT_all.T @ W_proj[rows_c]  [4096, 1024]  (row-parallel partial)
Host: out = sum_c part_c + b_proj;  k, v gathered from qkvT slices.

Matmuls run as fp32r (fp32 with 11-bit mantissa, 1 cyc/row) in a 3-term
hi/lo split (hi@hi + hi@lo + lo@hi) giving fp32-level precision.
"""

import numpy as np
from contextlib import ExitStack

import concourse.bass as bass
import concourse.mybir as mybir
import concourse.tile as tile
from concourse import bacc
from concourse.bass_utils import run_bass_kernel_spmd
from concourse.masks import make_identity

B, T, C = 2, 2048, 1024
H, D = 16, 64
N_CORES = 8
HPC = H // N_CORES          # heads per core
BT = B * T
NQ = 256                    # qkv gemm moving-tile (>=256 keeps fp32r at 1 cyc/row)
QT = 512                    # attention qt tile
KT = 128                    # attention kt block
NKT = T // KT               # 16
NQT = T // QT               # 4

r32 = mybir.dt.float32r
f32 = mybir.dt.float32
AOp = mybir.AluOpType

_CACHE = {}


def _round11(a):
    """fp32 -> fp32r (11 explicit mantissa bits, round to nearest even)."""
    b = a.view(np.uint32).astype(np.uint64)
    lsb = (b >> 12) & 1
    b = b + 0x7FF + lsb
    return (b & 0xFFFF_F000).astype(np.uint32).view(np.float32)


def _split(a):
    hi = _round11(a)
    return hi, (a - hi)


def _build():
    nc = bacc.Bacc("TRN2", target_bir_lowering=False, debug=False,
                   num_devices=N_CORES)

    xh_ap = nc.dram_tensor("xh", [C, BT], r32, kind="ExternalInput").ap()
    xl_ap = nc.dram_tensor("xl", [C, BT], r32, kind="ExternalInput").ap()
    wh_ap = nc.dram_tensor("wh", [C, 3 * HPC * D], r32, kind="ExternalInput").ap()
    wl_ap = nc.dram_tensor("wl", [C, 3 * HPC * D], r32, kind="ExternalInput").ap()
    bq_ap = nc.dram_tensor("bq", [3, 128], f32, kind="ExternalInput").ap()
    wph_ap = nc.dram_tensor("wph", [128, C], r32, kind="ExternalInput").ap()
    wpl_ap = nc.dram_tensor("wpl", [128, C], r32, kind="ExternalInput").ap()
    mk_ap = nc.dram_tensor("mk", [128, 896], f32, kind="ExternalInput").ap()
    oz_ap = nc.dram_tensor("oz", [2, 128, NKT], r32, kind="ExternalInput").ap()

    part_ap = nc.dram_tensor("part", [BT, C], f32, kind="ExternalOutput").ap()
    kv_ap = nc.dram_tensor("kv", [2, B, HPC, D, T], f32, kind="ExternalOutput").ap()

    with tile.TileContext(nc) as tc, ExitStack() as ctx:
        pool = lambda name, bufs, **kw: ctx.enter_context(
            tc.tile_pool(name=name, bufs=bufs, **kw))

        wpool = pool("w", 1)
        xpool = pool("x", 2)
        t1pool = pool("t1", 1)
        spool = pool("spl", 1)
        vpool = pool("vaug", 1)
        ppool = pool("p", 3)
        ypool = pool("y", 1)
        rpool = pool("r", 2)
        opool = pool("o", 2)
        cpool = pool("const", 1)

        ps_qkv = pool("ps_qkv", 2, space="PSUM")
        ps_s = pool("ps_s", 2, space="PSUM")
        ps_y = pool("ps_y", 1, space="PSUM")
        ps_m = pool("ps_m", 1, space="PSUM")

        # ---- resident constants ----
        wh_t, wl_t = [], []
        for k in range(C // 128):
            wt = wpool.tile([128, 3 * HPC * D], r32, tag=f"wh{k}")
            nc.sync.dma_start(wt[:], wh_ap[k * 128:(k + 1) * 128, :])
            wh_t.append(wt)
            wt = wpool.tile([128, 3 * HPC * D], r32, tag=f"wl{k}")
            nc.sync.dma_start(wt[:], wl_ap[k * 128:(k + 1) * 128, :])
            wl_t.append(wt)
        wph_t = wpool.tile([128, C], r32, tag="wph")
        nc.sync.dma_start(wph_t[:], wph_ap[:])
        wpl_t = wpool.tile([128, C], r32, tag="wpl")
        nc.sync.dma_start(wpl_t[:], wpl_ap[:])
        mask_t = cpool.tile([128, 896], f32, tag="mask")
        nc.sync.dma_start(mask_t[:], mk_ap[:])
        bias_t = cpool.tile([128, 3], f32, tag="bias")
        for m in range(3):
            nc.sync.dma_start(bias_t[:, m:m + 1],
                              bq_ap[m:m + 1, :].rearrange("o a -> a o"))
        ident = cpool.tile([128, 128], f32, tag="ident")
        make_identity(nc, ident[:])

        xh_r = xh_ap.rearrange("(k p) t -> p k t", p=128)
        xl_r = xl_ap.rearrange("(k p) t -> p k t", p=128)

        for b in range(B):
            # ---- QKV gemm for this batch: t1 = W_c.T @ xT + b ----
            t1q = t1pool.tile([128, T], f32, tag="t1q")
            t1k = t1pool.tile([128, T], f32, tag="t1k")
            t1v = t1pool.tile([128, T], f32, tag="t1v")
            t1 = [t1q, t1k, t1v]
            qh = spool.tile([128, T], r32, tag="qh")
            ql = spool.tile([128, T], r32, tag="ql")
            kh = spool.tile([128, T], r32, tag="kh")
            kl = spool.tile([128, T], r32, tag="kl")
            for n in range(T // NQ):
                col0 = b * T + n * NQ
                xh_tile = xpool.tile([128, C // 128, NQ], r32, tag="xh")
                nc.sync.dma_start(xh_tile[:], xh_r[:, :, col0:col0 + NQ])
                xl_tile = xpool.tile([128, C // 128, NQ], r32, tag="xl")
                nc.sync.dma_start(xl_tile[:], xl_r[:, :, col0:col0 + NQ])
                for m in range(3):
                    ps = ps_qkv.tile([128, NQ], f32, tag="ps_qkv")
                    msl = slice(m * 128, (m + 1) * 128)
                    i, last = 0, 3 * (C // 128) - 1
                    for k in range(C // 128):
                        for wt, xt in ((wh_t[k], xh_tile), (wh_t[k], xl_tile),
                                       (wl_t[k], xh_tile)):
                            nc.tensor.matmul(ps[:], wt[:, msl], xt[:, k, :],
                                             start=(i == 0), stop=(i == last))
                            i += 1
                    nc.vector.tensor_scalar_add(
                        t1[m][:, n * NQ:(n + 1) * NQ], ps[:], bias_t[:, m:m + 1])
                nsl = slice(n * NQ, (n + 1) * NQ)
                nc.vector.tensor_copy(qh[:, nsl], t1q[:, nsl])
                nc.vector.tensor_tensor(ql[:, nsl], t1q[:, nsl], qh[:, nsl],
                                        AOp.subtract)
                nc.vector.tensor_copy(kh[:, nsl], t1k[:, nsl])
                nc.vector.tensor_tensor(kl[:, nsl], t1k[:, nsl], kh[:, nsl],
                                        AOp.subtract)

            # ---- k, v DMA out (transposed layout; host untransposes) ----
            for hl in range(HPC):
                rsl = slice(hl * D, (hl + 1) * D)
                nc.sync.dma_start(kv_ap[0, b, hl], t1k[rsl, :])
                nc.sync.dma_start(kv_ap[1, b, hl], t1v[rsl, :])

            # ---- v natural (PE transpose) + ones column, hi/lo ----
            vh_aug = [vpool.tile([128, NKT, D + 1], r32, tag=f"vh{h}",
                                 name=f"vh{b}_{h}") for h in range(HPC)]
            vl_aug = [vpool.tile([128, NKT, D + 1], r32, tag=f"vl{h}",
                                 name=f"vl{b}_{h}") for h in range(HPC)]
            for h in range(HPC):
                nc.sync.dma_start(vh_aug[h][:, :, D:D + 1],
                                  oz_ap[0:1].rearrange("o p k -> p k o"))
                nc.sync.dma_start(vl_aug[h][:, :, D:D + 1],
                                  oz_ap[1:2].rearrange("o p k -> p k o"))
            for kt in range(NKT):
                tp = ps_m.tile([128, 128], f32, tag="tp")
                nc.tensor.transpose(tp[:], t1v[:, kt * 128:(kt + 1) * 128],
                                    ident[:])
                for h in range(HPC):
                    csl = slice(h * D, (h + 1) * D)
                    nc.vector.tensor_copy(vh_aug[h][:, kt, 0:D], tp[:, csl])
                    nc.vector.tensor_tensor(vl_aug[h][:, kt, 0:D], tp[:, csl],
                                            vh_aug[h][:, kt, 0:D], AOp.subtract)

            # ---- attention ----
            yh = ypool.tile([128, T], r32, tag="yh")
            yl = ypool.tile([128, T], r32, tag="yl")
            for qt in range(NQT):
                qtsl = slice(qt * QT, (qt + 1) * QT)
                psy = [ps_y.tile([D + 1, QT], f32, tag=f"psy{h}",
                                 name=f"psy{b}_{qt}_{h}") for h in range(HPC)]
                nkt = (qt + 1) * (QT // KT)
                for ktb in range(nkt):
                    delta = ktb * KT - qt * QT
                    ktsl = slice(ktb * KT, (ktb + 1) * KT)
                    pss = [ps_s.tile([128, QT], f32, tag="ps_s",
                                       name=f"pss{b}_{qt}_{ktb}_{h}")
                           for h in range(HPC)]
                    rsls = [slice(h * D, (h + 1) * D) for h in range(HPC)]
                    for a_, b_, st, sp_ in ((kh, qh, True, False),
                                            (kh, ql, False, False),
                                            (kl, qh, False, True)):
                        for h in range(HPC):
                            nc.tensor.matmul(pss[h][:], a_[rsls[h], ktsl],
                                             b_[rsls[h], qtsl],
                                             start=st, stop=sp_)
                    for h in range(HPC):
                        pf = ppool.tile([128, QT], f32, tag="pf")
                        nc.scalar.activation(pf[:], pss[h][:],
                                             mybir.ActivationFunctionType.Exp,
                                             scale=0.125)
                        if delta >= 0:  # partial (diagonal) block: apply mask
                            pm = ppool.tile([128, QT], f32, tag="pm")
                            nc.vector.tensor_tensor(
                                pm[:], pf[:],
                                mask_t[:, 384 - delta:384 - delta + QT],
                                AOp.mult)
                            pf = pm
                        phi = ppool.tile([128, QT], r32, tag="phi")
                        nc.vector.tensor_copy(phi[:], pf[:])
                        plo = ppool.tile([128, QT], r32, tag="plo")
                        nc.vector.tensor_tensor(plo[:], pf[:], phi[:],
                                                AOp.subtract)
                        first = ktb == 0
                        last = ktb == nkt - 1
                        nc.tensor.matmul(psy[h][:], vh_aug[h][:, ktb, :],
                                         phi[:], start=first, stop=False)
                        nc.tensor.matmul(psy[h][:], vl_aug[h][:, ktb, :],
                                         phi[:], start=False, stop=False)
                        nc.tensor.matmul(psy[h][:], vh_aug[h][:, ktb, :],
                                         plo[:], start=False, stop=last)
                # normalize: yT = y_raw / denom, split hi/lo
                for h in range(HPC):
                    rsl = slice(h * D, (h + 1) * D)
                    rc = rpool.tile([1, QT], f32, tag="rc")
                    nc.vector.reciprocal(rc[:], psy[h][D:D + 1, :])
                    rb = rpool.tile([D, QT], f32, tag="rb")
                    nc.gpsimd.partition_broadcast(rb[:], rc[:])
                    nc.vector.tensor_tensor(yh[rsl, qtsl], psy[h][0:D, :],
                                            rb[:], AOp.mult)
                    tmp = rpool.tile([128, QT], f32, tag="tmp")
                    nc.vector.tensor_tensor(tmp[rsl, :], psy[h][0:D, :], rb[:],
                                            AOp.mult)
                    nc.vector.tensor_tensor(yl[rsl, qtsl], tmp[rsl, :],
                                            yh[rsl, qtsl], AOp.subtract)

            # ---- output projection (row-parallel partial) ----
            for m in range(T // 128):
                msl = slice(m * 128, (m + 1) * 128)
                osl = slice(b * T + m * 128, b * T + (m + 1) * 128)
                for oc in range(C // 512):
                    ocsl = slice(oc * 512, (oc + 1) * 512)
                    ps = ps_m.tile([128, 512], f32, tag="ps_o")
                    nc.tensor.matmul(ps[:], yh[:, msl], wph_t[:, ocsl],
                                     start=True, stop=False)
                    nc.tensor.matmul(ps[:], yh[:, msl], wpl_t[:, ocsl],
                                     start=False, stop=False)
                    nc.tensor.matmul(ps[:], yl[:, msl], wph_t[:, ocsl],
                                     start=False, stop=True)
                    ot = opool.tile([128, 512], f32, tag="ot")
                    nc.vector.tensor_copy(ot[:], ps[:])
                    nc.sync.dma_start(part_ap[osl, ocsl], ot[:])

    nc.compile()
    return nc


def _get_nc():
    if "nc" not in _CACHE:
        _CACHE["nc"] = _build()
    return _CACHE["nc"]


def make_in_maps(x, W_kqv, b_kqv, W_proj, b_proj):
    x = np.ascontiguousarray(np.asarray(x, np.float32))
    W_kqv = np.asarray(W_kqv, np.float32)
    b_kqv = np.asarray(b_kqv, np.float32)
    W_proj = np.asarray(W_proj, np.float32)

    xT = np.ascontiguousarray(x.reshape(BT, C).T)
    xh, xl = _split(xT)

    # causal strip: mk[p, g] = 1.0 if p + 384 <= g
    mk = (np.arange(128)[:, None] + 384 <= np.arange(896)[None, :]).astype(np.float32)
    oz = np.stack([np.ones((128, NKT), np.float32),
                   np.zeros((128, NKT), np.float32)])

    in_maps = []
    for c in range(N_CORES):
        heads = [c * HPC + i for i in range(HPC)]
        cols = np.concatenate([
            np.arange(blk * C + h * D, blk * C + (h + 1) * D)
            for blk in range(3) for h in heads])
        w_c = np.ascontiguousarray(W_kqv[:, cols])
        wh, wl = _split(w_c)
        bq = np.ascontiguousarray(b_kqv[cols].reshape(3, 128))
        rows = np.concatenate([np.arange(h * D, (h + 1) * D) for h in heads])
        wp_c = np.ascontiguousarray(W_proj[rows, :])
        wph, wpl = _split(wp_c)
        in_maps.append({
            "xh": xh, "xl": xl, "wh": wh, "wl": wl, "bq": bq,
            "wph": wph, "wpl": wpl, "mk": mk, "oz": oz,
        })
    return in_maps


def assemble(results, b_proj):
    b_proj = np.asarray(b_proj, np.float32)
    parts = np.stack([results[c]["part"] for c in range(N_CORES)])
    out = (parts.sum(axis=0) + b_proj[None, :]).reshape(B, T, C)
    k = np.empty((B, H, T, D), np.float32)
    v = np.empty((B, H, T, D), np.float32)
    for c in range(N_CORES):
        kv = results[c]["kv"]          # [2, B, HPC, D, T]
        for hl in range(HPC):
            k[:, c * HPC + hl] = kv[0, :, hl].transpose(0, 2, 1)
            v[:, c * HPC + hl] = kv[1, :, hl].transpose(0, 2, 1)
    return out, k, v


def kernel(x, W_kqv, b_kqv, W_proj, b_proj):
    nc = _get_nc()
    in_maps = make_in_maps(x, W_kqv, b_kqv, W_proj, b_proj)
    res = run_bass_kernel_spmd(nc, in_maps, list(range(N_CORES)))
    return assemble(res.results, b_proj)
